# revision 3
# baseline (speedup 1.0000x reference)
"""Distributed kernel for nn_ApplyKernel (gnn_message_passing).

Math (reference):
    rel[z,a,b,:] = geometry[z,b,:] - geometry[z,a,:]
    feat = [rel, |rel|]                               # [z,a,b,4]
    h    = gelu(feat @ W1.T + b1)                     # [z,a,b,64]
    k    = (h @ W2.T + b2).reshape(z,a,b,16,16)
    out[z,a,b,i] = sum_j k[z,a,b,i,j] * features[z,b,j]

Factoring used here (exact, not an approximation):
    out[z,a,b,i] = sum_h h[z,a,b,h] * V[z,b,h,i] + c[z,b,i]
      V[z,b,h,i] = sum_j W2[i*16+j, h] * features[z,b,j]
      c[z,b,i]   = sum_j b2[i*16+j]   * features[z,b,j]
which removes the [z,a,b,256] intermediate entirely (12x fewer FLOPs).

Sharding: the query-point axis 'a' (512) is split across the 8 NeuronCores
(64 rows each); geometry/features/weights are replicated.

Wall-time structure (axon-tunneled devices, ~70ms dispatch latency,
~60-100MB/s D2H):
  - ONE cached jitted shard_map call per kernel() invocation
  - per-core output returned as fp16 (halves D2H bytes; rel tol is 2e-2)
  - D2H via a single jax.device_get on the 8 per-shard single-device
    arrays (concurrent transfers; ~2x faster than any other fetch path)
  - final assembly (concat + fp32 upcast) on host
"""

import numpy as np

B, N, C_IN, C_OUT, HID = 2, 512, 16, 16, 64
M = 8  # cores
A_SH = N // M  # 64 query rows per core


def _erf(x):
    # Abramowitz & Stegun 7.1.26, |err| < 1.5e-7 — plenty for the 2e-2 gate.
    sign = np.sign(x)
    x = np.abs(x)
    t = 1.0 / (1.0 + 0.3275911 * x)
    y = 1.0 - (((((1.061405429 * t - 1.453152027) * t) + 1.421413741) * t
                - 0.284496736) * t + 0.254829592) * t * np.exp(-x * x)
    return sign * y


def _numpy_fallback(features, geometry, W1, b1, W2, b2):
    W2r = W2.reshape(C_OUT, C_IN, HID)
    b2r = b2.reshape(C_OUT, C_IN)
    V = np.einsum("ijh,zbj->zbhi", W2r, features).astype(np.float32)
    c = np.einsum("ij,zbj->zbi", b2r, features).astype(np.float32)
    out = np.empty((B, N, N, C_OUT), dtype=np.float32)
    for z in range(B):
        for a0 in range(0, N, 64):
            ga = geometry[z, a0:a0 + 64]                       # [64, 3]
            rel = geometry[z][None, :, :] - ga[:, None, :]     # [64, N, 3]
            norm = np.sqrt(np.sum(rel * rel, -1, keepdims=True) + 1e-12)
            feat = np.concatenate([rel, norm], -1)             # [64, N, 4]
            p = feat @ W1.T + b1                               # [64, N, HID]
            h = 0.5 * p * (1.0 + _erf(p / np.sqrt(2.0, dtype=np.float32)))
            out[z, a0:a0 + 64] = np.einsum("abh,bhi->abi", h, V[z]) + c[z][None]
    return out


_CACHE = {}


def _get_jitted():
    """Build (once) the jitted 8-core sharded computation."""
    if "fn" in _CACHE:
        return _CACHE["fn"]

    import jax
    import jax.numpy as jnp
    from jax.sharding import Mesh, PartitionSpec as P
    try:
        from jax import shard_map
        def smap(f, mesh, in_specs, out_specs):
            return shard_map(f, mesh=mesh, in_specs=in_specs,
                             out_specs=out_specs, check_vma=False)
    except ImportError:
        from jax.experimental.shard_map import shard_map
        def smap(f, mesh, in_specs, out_specs):
            return shard_map(f, mesh=mesh, in_specs=in_specs,
                             out_specs=out_specs, check_rep=False)

    devices = jax.devices()
    if len(devices) < M:
        _CACHE["fn"] = None
        return None

    def core_fn(geo_a, geometry, features, W1, b1, W2r, b2r):
        rel = geometry[:, None, :, :] - geo_a[:, :, None, :]      # [z, A_SH, N, 3]
        norm = jnp.sqrt(jnp.sum(rel * rel, -1, keepdims=True) + 1e-12)
        feat = jnp.concatenate([rel, norm], -1)                   # [z, A_SH, N, 4]
        h = jax.nn.gelu(jnp.einsum("zabf,hf->zabh", feat, W1) + b1,
                        approximate=False)
        V = jnp.einsum("ijh,zbj->zbhi", W2r, features)
        c = jnp.einsum("ij,zbj->zbi", b2r, features)
        out = jnp.einsum("zabh,zbhi->zabi", h, V) + c[:, None, :, :]
        return out                                                # [z, A_SH, N, i]

    def body(geo_a, geometry, features, W1, b1, W2r, b2r):
        out = core_fn(geo_a[0], geometry, features, W1, b1, W2r, b2r)
        return out.astype(jnp.float16)[None]

    mesh = Mesh(np.asarray(devices[:M]), ("core",))
    fn = jax.jit(smap(body, mesh,
                      (P("core"),) + (P(),) * 6,
                      P("core")))
    _CACHE["fn"] = fn
    return fn


def _jax_compute(features, geometry, W1, b1, W2, b2):
    import jax

    fn = _get_jitted()
    if fn is None:
        return None
    W2r = W2.reshape(C_OUT, C_IN, HID)
    b2r = b2.reshape(C_OUT, C_IN)
    geo_a = np.ascontiguousarray(
        geometry.reshape(B, M, A_SH, 3).transpose(1, 0, 2, 3))
    r = fn(geo_a, geometry, features, W1, b1, W2r, b2r)
    shards = sorted(r.addressable_shards, key=lambda s: s.index[0].start)
    datas = jax.device_get([s.data for s in shards])   # 8 x [1, z, A_SH, N, i] fp16
    out = np.empty((B, N, N, C_OUT), dtype=np.float32)
    for d, piece in enumerate(datas):
        out[:, d * A_SH:(d + 1) * A_SH] = piece[0]     # fp16 -> fp32 on assign
    return out


def kernel(**inputs) -> np.ndarray:
    args = tuple(
        np.asarray(inputs[k], dtype=np.float32)
        for k in ("features", "geometry", "W1", "b1", "W2", "b2")
    )
    out = None
    try:
        import signal

        def _raise(*_a):
            raise TimeoutError("device path timed out")

        old = signal.signal(signal.SIGALRM, _raise)
        signal.alarm(600)
        try:
            out = _jax_compute(*args)
        finally:
            signal.alarm(0)
            signal.signal(signal.SIGALRM, old)
    except Exception:
        out = None
    if out is None or out.shape != (B, N, N, C_OUT) or not np.isfinite(out).all():
        out = _numpy_fallback(*args)
    return np.ascontiguousarray(out.astype(np.float32))


if __name__ == "__main__":
    rng = np.random.default_rng(0)
    ins = {
        "features": rng.standard_normal((B, N, C_IN), dtype=np.float32),
        "geometry": rng.standard_normal((B, N, 3), dtype=np.float32),
        "W1": rng.standard_normal((HID, 4), dtype=np.float32) * 0.5,
        "b1": rng.standard_normal((HID,), dtype=np.float32) * 0.1,
        "W2": rng.standard_normal((C_OUT * C_IN, HID), dtype=np.float32) * 0.1,
        "b2": rng.standard_normal((C_OUT * C_IN,), dtype=np.float32) * 0.1,
    }
    out = kernel(**ins)
    print(out.shape, out.dtype)
